# revision 29
# baseline (speedup 1.0000x reference)
"""Trainium2 Bass kernel for BidPrefix: per-row cumprod + 3-point gather.

Reference semantics (per row b of inputs [B, 302]):
  rates = inputs[b, :300]; bid = int(inputs[b, 300]); mp = int(inputs[b, 301])
  cpz[k] = prod(rates[:k]) (cpz[0] = 1)
  out[b] = [cpz[bid], cpz[mp+1], cpz[mp]]

Strategy: pure data parallel over 8 NeuronCores. Rows are host-sorted by
max(bid, mp) descending and packed 128-per-tile so every tap in tile t
lies below a per-tile bound L[t]. Tiles are grouped into chunks whose
page width W is quantized to 12 fixed levels; per chunk the host packs
two header blocks (bid[gsz], rates[mp][gsz]) and contiguous pages
[mp, 1.0, rates[0:W], 0.0] (N = W+3) in a flat [128, TOT] DRAM layout.

On device, ONE hand-written custom DVE op per W-level (MERGETAP{W}_ANT)
computes BOTH taps in a single pass over each page. Its 6-uop FSM
[seed; boundary; steadyA; steadyB; penultimate; last] runs, per page:
pgidx = -1,0,1,... (so the packed 1.0 gives cp[e] = cpz[e] exactly),
cp = running product, R1 += (pgidx==bid)*cp with bid streamed stride-0
from the header (Src1), and R2 += (pgidx==mp)*cp with mp captured from
the page's first cell into the stage-5 swap flop by the boundary uop.
The page interior is covered by two COUNT-repeat steady uops (the
repeat_cnt field is 8-bit, so W up to 300 is split in half); every
consuming uop writes (write-gated uops hang the engine), so the dst is a
stride-1 junk strip whose page slots N-2 and N-1 receive cpz[bid] (the
penultimate uop selects the R1 delay lane) and cpz[mp] (the last uop
selects ALU_OUT = R2). The trailing 0.0 pad cell guarantees both sums
are complete by slot N-2. Results leave via a strided [P, gsz, 2] DMA of
those slots plus cpz[mp+1] = cpz[mp] * rates[mp], one small Vector
multiply per chunk against the packed rates[mp] block (bit-exact with
the reference's sequential f32 cumprod). bid==0 / mp==0 fall out
naturally (cp[0] = 1). The host does layout only; every multiply happens
on device.
"""

import dataclasses
import sys

if "/opt/trn_rl_repo" not in sys.path:
    sys.path.insert(0, "/opt/trn_rl_repo")

import numpy as np

S = 300
COLS = 302
P = 128
NCORES = 8
TILES = 196
BPC = TILES * P  # 25088 rows per core
BTOT = 200000
BUDGET = 6144  # per-partition f32 slots per chunk
RAMP = [384, 768, 1536, 3072]

# quantized page-width levels (one custom op per level; 5-bit opcode-row
# budget allows 31 - 16 builtins = 15)
LEVELS = [300, 240, 192, 153, 122, 97, 77, 61, 48, 38, 30, 2]

TRACE = False
LAST_RESULTS = None

_MERGETAPS = None


def _get_mergetaps():
    """Register the merged two-tap page ops, one per W level (idempotent)."""
    global _MERGETAPS
    if _MERGETAPS is not None:
        return _MERGETAPS
    import concourse.dve_ops as dve_ops
    from concourse.dve_ops import OPS, DveOp
    from concourse.dve_spec import (
        AluOp, Bin, Scan, Spec, Src0, Src1, Zero, One, eq,
    )
    from concourse.dve_uop import (
        AluInp, DelayInp, DveOpSpec, InpSel, OutPath, OutSel, Trigger,
        UopConfig, UopDpConfig, ENABLE,
    )

    existing = {op.name: op for op in OPS}

    LSRC0, LSRC1, LONE, LA, LB, LC = 0, 1, 2, 3, 4, 5
    D = AluInp.PREV_DELAY_0

    def lane(i):
        return AluInp(int(D) + i)

    def _ref(in0, in1, s0, s1, imm2):
        x = in0.astype(np.float32)
        cpz = np.cumprod(x[..., 1:], axis=-1, dtype=np.float32)
        mp = x[..., 0].astype(np.int64)[..., None]
        bid = np.asarray(in1, np.float32)[..., 0].astype(np.int64)[..., None]
        out = np.zeros(x.shape, np.float32)
        out[..., -2] = np.take_along_axis(cpz, bid, axis=-1)[..., 0]
        out[..., -1] = np.take_along_axis(cpz, mp, axis=-1)[..., 0]
        return out

    def mk_steady():
        u = UopConfig()
        u.enable_input(InpSel.SRC_0, LSRC0 + 1)
        u.enable_input(InpSel.SRC_1, LSRC1 + 1)
        u.enable_input(InpSel.ONE_F32, LONE + 1)
        dp = u.datapath_config
        for st in range(8):
            dp[st].pass_through_delay(LSRC0, LSRC1, LONE, LA, LB, LC)
        # st0 pgidx; st1 cp (capture pgidx->A); st2 eq1 (capture cp->B);
        # st3 v1; st4 R1; st5 eq2 vs swap[mp] (capture R1->C); st6 v2;
        # st7 R2 — R1/R2 are CURR-feedback running sums
        dp[0].enable_alu(AluOp.ADD, AluInp.CURR_ALU_OUT, lane(LONE))
        dp[1].enable_alu(AluOp.MULTIPLY, AluInp.CURR_ALU_OUT, lane(LSRC0))
        dp[1].enable_delay_from_src(DelayInp.PREV_ALU_OUT, LA)
        dp[2].enable_alu(AluOp.IS_EQ, lane(LA), lane(LSRC1))
        dp[2].enable_delay_from_src(DelayInp.PREV_ALU_OUT, LB)
        dp[3].enable_alu(AluOp.MULTIPLY, AluInp.PREV_ALU_OUT, lane(LB))
        dp[4].enable_alu(AluOp.ADD, AluInp.CURR_ALU_OUT, AluInp.PREV_ALU_OUT)
        dp[5].enable_alu(AluOp.IS_EQ, lane(LA), AluInp.CURR_SWAP_OUT)
        dp[5].enable_delay_from_src(DelayInp.PREV_ALU_OUT, LC)
        dp[6].enable_alu(AluOp.MULTIPLY, AluInp.PREV_ALU_OUT, lane(LB))
        dp[7].enable_alu(AluOp.ADD, AluInp.CURR_ALU_OUT, AluInp.PREV_ALU_OUT)
        u.require_inp0 = ENABLE
        u.require_inp1 = ENABLE
        # every consuming uop must write (write-gated uops hang the engine)
        u.enable_output(OutSel.ALU_OUT, OutPath.WR0_LO)
        return u

    def mk_boundary():
        # first element of each page (the [mp] cell): reset the three
        # feedback flops and capture mp into st5's swap flop
        u = mk_steady()
        dp = u.datapath_config
        u.enable_input(InpSel.CONST_0, LC + 1)  # s0 immediate = -1.0
        dp[0].enable_alu(AluOp.BYPASS, lane(LC))
        dp[1].enable_alu(AluOp.BYPASS, lane(LONE))
        dp[4].enable_alu(AluOp.BYPASS, AluInp.PREV_ALU_OUT)
        dp[5] = UopDpConfig()
        dp[5].pass_through_delay(LSRC0, LSRC1, LONE, LA, LB, LC)
        dp[5].enable_alu(AluOp.BYPASS, lane(LSRC0))
        dp[5].swap_enable = ENABLE
        dp[6].enable_alu(AluOp.IS_EQ, lane(LA), AluInp.PREV_ALU_OUT)
        dp[7].enable_alu(AluOp.BYPASS, AluInp.PREV_ALU_OUT)
        u.repeat_count = 1
        u.trigger = (Trigger.SRC_TENSOR_DONE, Trigger.NONE, Trigger.COUNT)
        u.next_uop = (0, 0, 2)
        return u

    def mk_seed():
        # non-consuming entry: reset the feedback flops
        u = UopConfig()
        u.enable_input(InpSel.ONE_F32, LONE + 1)
        u.enable_input(InpSel.CONST_0, LC + 1)
        u.enable_input(InpSel.ZERO, LB + 1)
        dp = u.datapath_config
        for st in range(8):
            dp[st].pass_through_delay(LONE, LB, LC)
        dp[0].enable_alu(AluOp.BYPASS, lane(LC))
        dp[1].enable_alu(AluOp.BYPASS, lane(LONE))
        dp[4].enable_alu(AluOp.BYPASS, lane(LB))
        dp[7].enable_alu(AluOp.BYPASS, lane(LB))
        u.repeat_count = 1
        u.trigger = (Trigger.COUNT, Trigger.NONE, Trigger.NONE)
        u.next_uop = (1, 0, 0)
        return u

    def _uops(W, ver):
        # page N = W+3; boundary 1 elem + steadies W elems + pen 1 + last 1
        ra = (W + 1) // 2
        rb = W - ra
        assert 1 <= ra <= 255 and 1 <= rb <= 255, W
        seed = mk_seed()
        b1 = mk_boundary()
        stA = mk_steady()
        stA.repeat_count = ra
        stA.trigger = (Trigger.SRC_TENSOR_DONE, Trigger.NONE, Trigger.COUNT)
        stA.next_uop = (0, 0, 3)
        stB = mk_steady()
        stB.repeat_count = rb
        stB.trigger = (Trigger.SRC_TENSOR_DONE, Trigger.NONE, Trigger.COUNT)
        stB.next_uop = (0, 0, 4)
        pen = mk_steady()
        pen.repeat_count = 1
        pen.trigger = (Trigger.SRC_TENSOR_DONE, Trigger.NONE, Trigger.COUNT)
        pen.next_uop = (0, 0, 5)
        pen.out[OutPath.WR0_LO] = OutSel(int(OutSel.DELAY_0) + LC)  # R1
        last = mk_steady()
        last.trigger = (
            Trigger.SRC_TENSOR_DONE, Trigger.SUB_DIM_DONE, Trigger.NONE,
        )
        last.next_uop = (0, 1, 0)
        uops = [seed, b1, stA, stB, pen, last]
        for u in uops:
            u.validate(ver)
        return uops

    @dataclasses.dataclass(frozen=True)
    class _RawDveOp(DveOp):
        raw_uops: dict = dataclasses.field(
            default_factory=dict, compare=False, hash=False
        )

        def compile(self, ver):
            sp = DveOpSpec(
                name=self.name,
                opcode=dve_ops.get_dve_sub_opcode(self.name),
                uops=self.raw_uops[ver],
                rd1_en=True,
            )
            sp.validate(ver)
            return sp

    spec_body = eq(
        Scan(AluOp.ADD, One, init=Bin(AluOp.SUBTRACT, Zero, One)), Src1
    ) * Scan(AluOp.MULTIPLY, Src0, init=One)

    taps = {}
    for W in LEVELS:
        name = f"MERGETAP{W}_ANT"
        if name in existing:
            taps[W] = existing[name]
            continue
        spec = Spec(body=spec_body, reference=_ref)
        raw = {ver: _uops(W, ver) for ver in ("v3", "v4")}
        shas = {
            ver: DveOpSpec(name=name, opcode=0, uops=u, rd1_en=True).sha(ver)
            for ver, u in raw.items()
        }
        op = _RawDveOp(name, spec, subdim=True, uops_sha=shas, raw_uops=raw)
        OPS.append(op)
        dve_ops._SUB_OPCODE_FOR_NAME[name] = (
            dve_ops._CUSTOM_DVE_ROW_BASE + len(OPS) - 1
        )
        dve_ops.CUSTOM_DVE_SPECS[name] = spec
        taps[W] = op
    _MERGETAPS = taps
    return taps


def _emit_mergetap(nc, op, out0, in0, in1):
    import concourse.bass_isa as bass_isa
    import concourse.mybir as mybir
    from concourse.dve_ops import get_dve_sub_opcode

    v = nc.vector
    if op.name not in nc.m.ant_custom_dve_ops:
        nc.m.ant_custom_dve_ops = sorted({*nc.m.ant_custom_dve_ops, op.name})
    shape = bass_isa.CustomDveShape.STT
    isa_opcode = nc.isa.Opcode[
        f"NEURON_ISA_TPB_OPCODE_CUSTOM_DVE_ANT_{shape.slot()}"
    ].value
    ins = [
        v.lower_ap(in0, for_isa=True, opt=False),
        v.lower_ap(in1, for_isa=True, opt=False),
        mybir.ImmediateValue(dtype=mybir.dt.float32, value=-1.0),
        mybir.ImmediateValue(dtype=mybir.dt.float32, value=0.0),
    ]
    outs = [v.lower_ap(out0, for_isa=True, opt=False)]
    return v.add_instruction(
        bass_isa.InstCustomDveAnt(
            name=nc.get_next_instruction_name(),
            op_name=op.name,
            rd1_en=True,
            subdim=0x02,
            imm2=0.0,
            shape=shape,
            row=get_dve_sub_opcode(op.name),
            isa_opcode=isa_opcode,
            ins=ins,
            outs=outs,
        )
    )


def _plan_groups(L_list):
    """Chunks of tiles sharing one quantized page width W: walk tiles
    (sorted L desc), W = smallest level >= L[t0]; fill until the budget.
    Small ramp-up budgets let the DVE start early."""
    n = len(L_list)
    groups = []
    t0 = 0
    gi = 0
    while t0 < n:
        budget = RAMP[gi] if gi < len(RAMP) else BUDGET
        W = min(lv for lv in LEVELS if lv >= max(int(L_list[t0]), 1))
        gsz = max(1, budget // (W + 5))
        gsz = min(gsz, n - t0)
        # all tiles in the chunk must fit level W
        for j in range(1, gsz):
            if L_list[t0 + j] > W:
                gsz = j
                break
        rem = n - t0 - gsz
        if 0 < rem < 3:
            gsz = max(1, gsz - (3 - rem))
        groups.append((t0, gsz, W))
        t0 += gsz
        gi += 1
    t0, gsz, W = groups[-1]
    if gsz >= 12:
        groups[-1] = (t0, gsz - 8, W)
        W2 = min(lv for lv in LEVELS if lv >= max(int(L_list[t0 + gsz - 8]), 1))
        W3 = min(lv for lv in LEVELS if lv >= max(int(L_list[t0 + gsz - 2]), 1))
        groups.append((t0 + gsz - 8, 6, W2))
        groups.append((t0 + gsz - 2, 2, W3))
    return groups


def _group_cols(gsz, W):
    """Per-partition f32 slots: 2 header blocks (bid, rates[mp]) +
    contiguous pages [mp, 1.0, rates[0:W], 0.0]."""
    return 2 * gsz + gsz * (W + 3)


def build_nc(L_list, groups=None):
    import concourse.bacc as bacc
    import concourse.mybir as mybir
    from concourse import tile

    f32 = mybir.dt.float32
    A = mybir.AluOpType
    TAPS = _get_mergetaps()

    if groups is None:
        groups = _plan_groups(L_list)
    ntiles = len(L_list)
    offs = [0]
    for _, gsz, W in groups:
        offs.append(offs[-1] + _group_cols(gsz, W))

    nc = bacc.Bacc("TRN2", target_bir_lowering=False, debug=False)
    inp = nc.dram_tensor("inp", [P, offs[-1]], f32, kind="ExternalInput")
    out = nc.dram_tensor("out", [P, ntiles * 3], f32, kind="ExternalOutput")
    vin = inp.ap()
    vout = out.ap()

    with tile.TileContext(nc) as tc:
        with (
            tc.tile_pool(name="raw", bufs=4) as rawp,
            tc.tile_pool(name="junk", bufs=3) as junkp,
            tc.tile_pool(name="res", bufs=4) as resp,
        ):
            prepped = {}

            def prep(gj):
                _, gsz, W = groups[gj]
                g = rawp.tile([P, _group_cols(gsz, W)], f32, tag="raw")
                nc.sync.dma_start(g, vin[:, offs[gj] : offs[gj + 1]])
                prepped[gj] = g

            for gj in range(min(4, len(groups))):
                prep(gj)
            for gi, (t0, gsz, W) in enumerate(groups):
                if gi + 4 < len(groups):
                    prep(gi + 4)
                N = W + 3
                g = prepped.pop(gi)
                pages = g[:, 2 * gsz :].rearrange("p (s w) -> p s w", w=N)
                junk = junkp.tile([P, gsz * N], f32, tag="junk")
                j3 = junk.rearrange("p (s w) -> p s w", w=N)
                _emit_mergetap(
                    nc, TAPS[W],
                    out0=j3,
                    in0=pages,
                    in1=g[:, 0:gsz].unsqueeze(2).broadcast_to([P, gsz, N]),
                )
                # cpz[mp+1] = cpz[mp] * rates[mp] (header block)
                res = resp.tile([P, gsz], f32, tag="res")
                nc.vector.tensor_tensor(
                    res, j3[:, :, N - 1], g[:, gsz : 2 * gsz], A.mult
                )
                nc.scalar.dma_start(
                    vout[:, 3 * t0 : 3 * t0 + 2 * gsz], j3[:, :, N - 2 : N]
                )
                nc.scalar.dma_start(
                    vout[:, 3 * t0 + 2 * gsz : 3 * (t0 + gsz)], res
                )

    nc.compile()
    return nc


def _prepare(x, ncores, tiles):
    """Sort rows by max(bid, mp) desc, pack into per-core flat page layout.

    Returns (arrs [ncores, P, TOT], L_list, groups, src_cpt)."""
    bpc = tiles * P
    npad = bpc * ncores - x.shape[0]
    assert npad >= 0
    if npad:
        padrows = np.zeros((npad, COLS), dtype=np.float32)
        padrows[:, :S] = 1.0
        xp = np.concatenate([x, padrows], axis=0)
    else:
        xp = x

    key = np.maximum(xp[:, S], xp[:, S + 1]).astype(np.int64)
    order = np.argsort(-key, kind="stable")
    nblocks = ncores * tiles
    src = order.reshape(nblocks, P).reshape(tiles, ncores, P)
    src_cpt = np.ascontiguousarray(src.transpose(1, 2, 0))  # [core, p, t]

    block_max = key[order].reshape(nblocks, P)[:, 0]
    L_list = np.maximum(block_max.reshape(tiles, ncores).max(axis=1), 1)
    L_list = [int(v) for v in L_list]
    groups = _plan_groups(L_list)

    rows = xp[src_cpt]  # [ncores, P, tiles, COLS]
    parts = []
    for t0, gsz, W in groups:
        rg = rows[:, :, t0 : t0 + gsz, :]
        hdr = np.empty((ncores, P, 2, gsz), dtype=np.float32)
        hdr[:, :, 0] = rg[..., S]  # bid
        mp_i = rg[..., S + 1].astype(np.int64)[..., None]
        hdr[:, :, 1] = np.take_along_axis(rg[..., :S], mp_i, axis=-1)[..., 0]
        pg = np.empty((ncores, P, gsz, W + 3), dtype=np.float32)
        pg[..., 0] = rg[..., S + 1]  # mp
        pg[..., 1] = 1.0
        pg[..., 2 : 2 + W] = rg[..., :W]
        pg[..., W + 2] = 0.0
        parts.append(hdr.reshape(ncores, P, 2 * gsz))
        parts.append(pg.reshape(ncores, P, gsz * (W + 3)))
    arrs = np.concatenate(parts, axis=2)
    return np.ascontiguousarray(arrs), L_list, groups, src_cpt


_NC_CACHE = {}


def _get_nc(L_list, groups):
    key = tuple(groups)
    if key not in _NC_CACHE:
        _NC_CACHE[key] = build_nc(L_list, groups)
    return _NC_CACHE[key]


def _unpack_core(yc, groups):
    """[P, 3*TILES] device layout -> [P, tiles, 3] reference layout."""
    ntiles = sum(g[1] for g in groups)
    yt = np.empty((P, ntiles, 3), np.float32)
    for t0, gsz, W in groups:
        pairs = yc[:, 3 * t0 : 3 * t0 + 2 * gsz].reshape(P, gsz, 2)
        yt[:, t0 : t0 + gsz, 0] = pairs[..., 0]  # cpz[bid]
        yt[:, t0 : t0 + gsz, 2] = pairs[..., 1]  # cpz[mp]
        yt[:, t0 : t0 + gsz, 1] = yc[:, 3 * t0 + 2 * gsz : 3 * (t0 + gsz)]
    return yt


def kernel(inputs):
    global LAST_RESULTS
    x = np.ascontiguousarray(np.asarray(inputs), dtype=np.float32)
    assert x.shape == (BTOT, COLS), x.shape

    arrs, L_list, groups, src_cpt = _prepare(x, NCORES, TILES)
    in_maps = [{"inp": np.ascontiguousarray(arrs[c])} for c in range(NCORES)]

    nc = _get_nc(L_list, groups)
    from concourse.bass_utils import run_bass_kernel_spmd

    r = run_bass_kernel_spmd(
        nc, in_maps, core_ids=list(range(NCORES)), trace=TRACE
    )
    LAST_RESULTS = r
    ys = np.stack(
        [
            _unpack_core(np.asarray(r.results[c]["out"]), groups)
            for c in range(NCORES)
        ]
    )
    out = np.empty((NCORES * BPC, 3), dtype=np.float32)
    out[src_cpt.reshape(-1)] = ys.reshape(-1, 3)
    return np.ascontiguousarray(out[:BTOT])


# revision 30
# speedup vs baseline: 2.1439x; 2.1439x over previous
"""Trainium2 Bass kernel for BidPrefix: per-row cumprod + 3-point gather.

Reference semantics (per row b of inputs [B, 302]):
  rates = inputs[b, :300]; bid = int(inputs[b, 300]); mp = int(inputs[b, 301])
  cpz[k] = prod(rates[:k]) (cpz[0] = 1)
  out[b] = [cpz[bid], cpz[mp+1], cpz[mp]]

Strategy: pure data parallel over 8 NeuronCores. Rows are host-sorted by
max(bid, mp) descending and packed 128-per-tile so every tap in tile t
lies below a per-tile bound L[t]. Tiles are grouped into chunks whose
page width W is quantized to 12 fixed levels; per chunk the host packs
two header blocks (bid[gsz], rates[mp][gsz]) and contiguous pages
[mp, 1.0, rates[0:W], 0.0] (N = W+3) in a flat [128, TOT] DRAM layout.

On device, ONE hand-written custom DVE op per W-level (MERGETAP{W}_ANT)
computes BOTH taps in a single pass over each page. Its 6-uop FSM
[seed; boundary; steadyA; steadyB; penultimate; last] runs, per page:
pgidx = -1,0,1,... (so the packed 1.0 gives cp[e] = cpz[e] exactly),
cp = running product, R1 += (pgidx==bid)*cp with bid streamed stride-0
from the header (Src1), and R2 += (pgidx==mp)*cp with mp captured from
the page's first cell into the stage-5 swap flop by the boundary uop.
The page interior is covered by two COUNT-repeat steady uops (the
repeat_cnt field is 8-bit, so W up to 300 is split in half); every
consuming uop writes (write-gated uops hang the engine), so the dst is a
stride-1 junk strip whose page slots N-2 and N-1 receive cpz[bid] (the
penultimate uop selects the R1 delay lane) and cpz[mp] (the last uop
selects ALU_OUT = R2). The trailing 0.0 pad cell guarantees both sums
are complete by slot N-2. Results leave via a strided [P, gsz, 2] DMA of
those slots plus cpz[mp+1] = cpz[mp] * rates[mp], one small Vector
multiply per chunk against the packed rates[mp] block (bit-exact with
the reference's sequential f32 cumprod). bid==0 / mp==0 fall out
naturally (cp[0] = 1). The host does layout only; every multiply happens
on device.
"""

import dataclasses
import sys

if "/opt/trn_rl_repo" not in sys.path:
    sys.path.insert(0, "/opt/trn_rl_repo")

import numpy as np

S = 300
COLS = 302
P = 128
NCORES = 8
TILES = 196
BPC = TILES * P  # 25088 rows per core
BTOT = 200000
BUDGET = 6144  # per-partition f32 slots per chunk
RAMP = [384, 768, 1536, 3072]

# quantized page-width levels (one custom op per level; 5-bit opcode-row
# budget allows 31 - 16 builtins = 15)
LEVELS = [300, 240, 192, 153, 122, 97, 77, 61, 48, 38, 30, 2]

TRACE = False
LAST_RESULTS = None

_MERGETAPS = None


def _get_mergetaps():
    """Register the merged two-tap page ops, one per W level (idempotent)."""
    global _MERGETAPS
    if _MERGETAPS is not None:
        return _MERGETAPS
    import concourse.dve_ops as dve_ops
    from concourse.dve_ops import OPS, DveOp
    from concourse.dve_spec import (
        AluOp, Bin, Scan, Spec, Src0, Src1, Zero, One, eq,
    )
    from concourse.dve_uop import (
        AluInp, DelayInp, DveOpSpec, InpSel, OutPath, OutSel, Trigger,
        UopConfig, UopDpConfig, ENABLE,
    )

    existing = {op.name: op for op in OPS}

    LSRC0, LSRC1, LONE, LA, LB, LC = 0, 1, 2, 3, 4, 5
    D = AluInp.PREV_DELAY_0

    def lane(i):
        return AluInp(int(D) + i)

    def _ref(in0, in1, s0, s1, imm2):
        x = in0.astype(np.float32)
        cpz = np.cumprod(x[..., 1:], axis=-1, dtype=np.float32)
        mp = x[..., 0].astype(np.int64)[..., None]
        bid = np.asarray(in1, np.float32)[..., 0].astype(np.int64)[..., None]
        out = np.zeros(x.shape, np.float32)
        out[..., -2] = np.take_along_axis(cpz, bid, axis=-1)[..., 0]
        out[..., -1] = np.take_along_axis(cpz, mp, axis=-1)[..., 0]
        return out

    def mk_steady():
        u = UopConfig()
        u.enable_input(InpSel.SRC_0, LSRC0 + 1)
        u.enable_input(InpSel.SRC_1, LSRC1 + 1)
        u.enable_input(InpSel.ONE_F32, LONE + 1)
        dp = u.datapath_config
        for st in range(8):
            dp[st].pass_through_delay(LSRC0, LSRC1, LONE, LA, LB, LC)
        # st0 pgidx; st1 cp (capture pgidx->A); st2 eq1 (capture cp->B);
        # st3 v1; st4 R1; st5 eq2 vs swap[mp] (capture R1->C); st6 v2;
        # st7 R2 — R1/R2 are CURR-feedback running sums
        dp[0].enable_alu(AluOp.ADD, AluInp.CURR_ALU_OUT, lane(LONE))
        dp[1].enable_alu(AluOp.MULTIPLY, AluInp.CURR_ALU_OUT, lane(LSRC0))
        dp[1].enable_delay_from_src(DelayInp.PREV_ALU_OUT, LA)
        dp[2].enable_alu(AluOp.IS_EQ, lane(LA), lane(LSRC1))
        dp[2].enable_delay_from_src(DelayInp.PREV_ALU_OUT, LB)
        dp[3].enable_alu(AluOp.MULTIPLY, AluInp.PREV_ALU_OUT, lane(LB))
        dp[4].enable_alu(AluOp.ADD, AluInp.CURR_ALU_OUT, AluInp.PREV_ALU_OUT)
        dp[5].enable_alu(AluOp.IS_EQ, lane(LA), AluInp.CURR_SWAP_OUT)
        dp[5].enable_delay_from_src(DelayInp.PREV_ALU_OUT, LC)
        dp[6].enable_alu(AluOp.MULTIPLY, AluInp.PREV_ALU_OUT, lane(LB))
        dp[7].enable_alu(AluOp.ADD, AluInp.CURR_ALU_OUT, AluInp.PREV_ALU_OUT)
        u.require_inp0 = ENABLE
        u.require_inp1 = ENABLE
        # every consuming uop must write (write-gated uops hang the engine)
        u.enable_output(OutSel.ALU_OUT, OutPath.WR0_LO)
        return u

    def mk_boundary():
        # first element of each page (the [mp] cell): reset the three
        # feedback flops and capture mp into st5's swap flop
        u = mk_steady()
        dp = u.datapath_config
        u.enable_input(InpSel.CONST_0, LC + 1)  # s0 immediate = -1.0
        dp[0].enable_alu(AluOp.BYPASS, lane(LC))
        dp[1].enable_alu(AluOp.BYPASS, lane(LONE))
        dp[4].enable_alu(AluOp.BYPASS, AluInp.PREV_ALU_OUT)
        dp[5] = UopDpConfig()
        dp[5].pass_through_delay(LSRC0, LSRC1, LONE, LA, LB, LC)
        dp[5].enable_alu(AluOp.BYPASS, lane(LSRC0))
        dp[5].swap_enable = ENABLE
        dp[6].enable_alu(AluOp.IS_EQ, lane(LA), AluInp.PREV_ALU_OUT)
        dp[7].enable_alu(AluOp.BYPASS, AluInp.PREV_ALU_OUT)
        u.repeat_count = 1
        u.trigger = (Trigger.SRC_TENSOR_DONE, Trigger.NONE, Trigger.COUNT)
        u.next_uop = (0, 0, 2)
        return u

    def mk_seed():
        # non-consuming entry: reset the feedback flops
        u = UopConfig()
        u.enable_input(InpSel.ONE_F32, LONE + 1)
        u.enable_input(InpSel.CONST_0, LC + 1)
        u.enable_input(InpSel.ZERO, LB + 1)
        dp = u.datapath_config
        for st in range(8):
            dp[st].pass_through_delay(LONE, LB, LC)
        dp[0].enable_alu(AluOp.BYPASS, lane(LC))
        dp[1].enable_alu(AluOp.BYPASS, lane(LONE))
        dp[4].enable_alu(AluOp.BYPASS, lane(LB))
        dp[7].enable_alu(AluOp.BYPASS, lane(LB))
        u.repeat_count = 1
        u.trigger = (Trigger.COUNT, Trigger.NONE, Trigger.NONE)
        u.next_uop = (1, 0, 0)
        return u

    def _uops(W, ver):
        # page N = W+3; boundary 1 elem + steadies W elems + pen 1 + last 1
        ra = (W + 1) // 2
        rb = W - ra
        assert 1 <= ra <= 255 and 1 <= rb <= 255, W
        seed = mk_seed()
        b1 = mk_boundary()
        stA = mk_steady()
        stA.repeat_count = ra
        stA.trigger = (Trigger.SRC_TENSOR_DONE, Trigger.NONE, Trigger.COUNT)
        stA.next_uop = (0, 0, 3)
        stB = mk_steady()
        stB.repeat_count = rb
        stB.trigger = (Trigger.SRC_TENSOR_DONE, Trigger.NONE, Trigger.COUNT)
        stB.next_uop = (0, 0, 4)
        pen = mk_steady()
        pen.repeat_count = 1
        pen.trigger = (Trigger.SRC_TENSOR_DONE, Trigger.NONE, Trigger.COUNT)
        pen.next_uop = (0, 0, 5)
        pen.out[OutPath.WR0_LO] = OutSel(int(OutSel.DELAY_0) + LC)  # R1
        last = mk_steady()
        last.trigger = (
            Trigger.SRC_TENSOR_DONE, Trigger.SUB_DIM_DONE, Trigger.NONE,
        )
        last.next_uop = (0, 1, 0)
        uops = [seed, b1, stA, stB, pen, last]
        for u in uops:
            u.validate(ver)
        return uops

    @dataclasses.dataclass(frozen=True)
    class _RawDveOp(DveOp):
        raw_uops: dict = dataclasses.field(
            default_factory=dict, compare=False, hash=False
        )

        def compile(self, ver):
            sp = DveOpSpec(
                name=self.name,
                opcode=dve_ops.get_dve_sub_opcode(self.name),
                uops=self.raw_uops[ver],
                rd1_en=True,
            )
            sp.validate(ver)
            return sp

    spec_body = eq(
        Scan(AluOp.ADD, One, init=Bin(AluOp.SUBTRACT, Zero, One)), Src1
    ) * Scan(AluOp.MULTIPLY, Src0, init=One)

    taps = {}
    for W in LEVELS:
        name = f"MERGETAP{W}_ANT"
        if name in existing:
            taps[W] = existing[name]
            continue
        spec = Spec(body=spec_body, reference=_ref)
        raw = {ver: _uops(W, ver) for ver in ("v3", "v4")}
        shas = {
            ver: DveOpSpec(name=name, opcode=0, uops=u, rd1_en=True).sha(ver)
            for ver, u in raw.items()
        }
        op = _RawDveOp(name, spec, subdim=True, uops_sha=shas, raw_uops=raw)
        OPS.append(op)
        dve_ops._SUB_OPCODE_FOR_NAME[name] = (
            dve_ops._CUSTOM_DVE_ROW_BASE + len(OPS) - 1
        )
        dve_ops.CUSTOM_DVE_SPECS[name] = spec
        taps[W] = op
    _MERGETAPS = taps
    return taps


def _emit_mergetap(nc, op, out0, in0, in1):
    import concourse.bass_isa as bass_isa
    import concourse.mybir as mybir
    from concourse.dve_ops import get_dve_sub_opcode

    v = nc.vector
    if op.name not in nc.m.ant_custom_dve_ops:
        nc.m.ant_custom_dve_ops = sorted({*nc.m.ant_custom_dve_ops, op.name})
    shape = bass_isa.CustomDveShape.STT
    isa_opcode = nc.isa.Opcode[
        f"NEURON_ISA_TPB_OPCODE_CUSTOM_DVE_ANT_{shape.slot()}"
    ].value
    ins = [
        v.lower_ap(in0, for_isa=True, opt=False),
        v.lower_ap(in1, for_isa=True, opt=False),
        mybir.ImmediateValue(dtype=mybir.dt.float32, value=-1.0),
        mybir.ImmediateValue(dtype=mybir.dt.float32, value=0.0),
    ]
    outs = [v.lower_ap(out0, for_isa=True, opt=False)]
    return v.add_instruction(
        bass_isa.InstCustomDveAnt(
            name=nc.get_next_instruction_name(),
            op_name=op.name,
            rd1_en=True,
            subdim=0x02,
            imm2=0.0,
            shape=shape,
            row=get_dve_sub_opcode(op.name),
            isa_opcode=isa_opcode,
            ins=ins,
            outs=outs,
        )
    )


def _plan_groups(L_list):
    """Chunks of tiles sharing one quantized page width W: walk tiles
    (sorted L desc), W = smallest level >= L[t0]; fill until the budget.
    Small ramp-up budgets let the DVE start early."""
    n = len(L_list)
    groups = []
    t0 = 0
    gi = 0
    while t0 < n:
        budget = RAMP[gi] if gi < len(RAMP) else BUDGET
        W = min(lv for lv in LEVELS if lv >= max(int(L_list[t0]), 1))
        gsz = max(1, budget // (W + 5))
        gsz = min(gsz, n - t0)
        # all tiles in the chunk must fit level W
        for j in range(1, gsz):
            if L_list[t0 + j] > W:
                gsz = j
                break
        rem = n - t0 - gsz
        if 0 < rem < 3:
            gsz = max(1, gsz - (3 - rem))
        groups.append((t0, gsz, W))
        t0 += gsz
        gi += 1
    t0, gsz, W = groups[-1]
    if gsz >= 12:
        groups[-1] = (t0, gsz - 8, W)
        W2 = min(lv for lv in LEVELS if lv >= max(int(L_list[t0 + gsz - 8]), 1))
        W3 = min(lv for lv in LEVELS if lv >= max(int(L_list[t0 + gsz - 2]), 1))
        groups.append((t0 + gsz - 8, 6, W2))
        groups.append((t0 + gsz - 2, 2, W3))
    return groups


def _group_cols(gsz, W):
    """Per-partition f32 slots: 2 header blocks (bid, rates[mp]) +
    contiguous pages [mp, 1.0, rates[0:W], 0.0]."""
    return 2 * gsz + gsz * (W + 3)


def build_nc(L_list, groups=None):
    import concourse.bacc as bacc
    import concourse.mybir as mybir
    from concourse import tile

    f32 = mybir.dt.float32
    A = mybir.AluOpType
    TAPS = _get_mergetaps()

    if groups is None:
        groups = _plan_groups(L_list)
    ntiles = len(L_list)
    offs = [0]
    for _, gsz, W in groups:
        offs.append(offs[-1] + _group_cols(gsz, W))

    nc = bacc.Bacc("TRN2", target_bir_lowering=False, debug=False)
    inp = nc.dram_tensor("inp", [P, offs[-1]], f32, kind="ExternalInput")
    out = nc.dram_tensor("out", [P, ntiles * 3], f32, kind="ExternalOutput")
    vin = inp.ap()
    vout = out.ap()

    with tile.TileContext(nc) as tc:
        with (
            tc.tile_pool(name="raw", bufs=4) as rawp,
            tc.tile_pool(name="junk", bufs=3) as junkp,
            tc.tile_pool(name="res", bufs=4) as resp,
        ):
            prepped = {}

            def prep(gj):
                _, gsz, W = groups[gj]
                g = rawp.tile([P, _group_cols(gsz, W)], f32, tag="raw")
                nc.sync.dma_start(g, vin[:, offs[gj] : offs[gj + 1]])
                prepped[gj] = g

            for gj in range(min(4, len(groups))):
                prep(gj)
            for gi, (t0, gsz, W) in enumerate(groups):
                if gi + 4 < len(groups):
                    prep(gi + 4)
                N = W + 3
                g = prepped.pop(gi)
                pages = g[:, 2 * gsz :].rearrange("p (s w) -> p s w", w=N)
                junk = junkp.tile([P, gsz * N], f32, tag="junk")
                j3 = junk.rearrange("p (s w) -> p s w", w=N)
                _emit_mergetap(
                    nc, TAPS[W],
                    out0=j3,
                    in0=pages,
                    in1=g[:, 0:gsz].unsqueeze(2).broadcast_to([P, gsz, N]),
                )
                # compact the tap pairs into contiguous SBUF (a strided
                # [P, gsz, 2] DMA straight from the junk strip generates
                # 8-byte scatter descriptors at ~6us per transfer), then
                # cpz[mp+1] = cpz[mp] * rates[mp], then ONE contiguous DMA
                res = resp.tile([P, 3 * gsz], f32, tag="res")
                nc.vector.tensor_copy(
                    res[:, 0 : 2 * gsz].rearrange("p (s k) -> p s k", k=2),
                    j3[:, :, N - 2 : N],
                )
                nc.vector.tensor_tensor(
                    res[:, 2 * gsz :], j3[:, :, N - 1], g[:, gsz : 2 * gsz],
                    A.mult,
                )
                nc.scalar.dma_start(vout[:, 3 * t0 : 3 * (t0 + gsz)], res)

    nc.compile()
    return nc


def _prepare(x, ncores, tiles):
    """Sort rows by max(bid, mp) desc, pack into per-core flat page layout.

    Returns (arrs [ncores, P, TOT], L_list, groups, src_cpt)."""
    bpc = tiles * P
    npad = bpc * ncores - x.shape[0]
    assert npad >= 0
    if npad:
        padrows = np.zeros((npad, COLS), dtype=np.float32)
        padrows[:, :S] = 1.0
        xp = np.concatenate([x, padrows], axis=0)
    else:
        xp = x

    key = np.maximum(xp[:, S], xp[:, S + 1]).astype(np.int64)
    order = np.argsort(-key, kind="stable")
    nblocks = ncores * tiles
    src = order.reshape(nblocks, P).reshape(tiles, ncores, P)
    src_cpt = np.ascontiguousarray(src.transpose(1, 2, 0))  # [core, p, t]

    block_max = key[order].reshape(nblocks, P)[:, 0]
    L_list = np.maximum(block_max.reshape(tiles, ncores).max(axis=1), 1)
    L_list = [int(v) for v in L_list]
    groups = _plan_groups(L_list)

    rows = xp[src_cpt]  # [ncores, P, tiles, COLS]
    parts = []
    for t0, gsz, W in groups:
        rg = rows[:, :, t0 : t0 + gsz, :]
        hdr = np.empty((ncores, P, 2, gsz), dtype=np.float32)
        hdr[:, :, 0] = rg[..., S]  # bid
        mp_i = rg[..., S + 1].astype(np.int64)[..., None]
        hdr[:, :, 1] = np.take_along_axis(rg[..., :S], mp_i, axis=-1)[..., 0]
        pg = np.empty((ncores, P, gsz, W + 3), dtype=np.float32)
        pg[..., 0] = rg[..., S + 1]  # mp
        pg[..., 1] = 1.0
        pg[..., 2 : 2 + W] = rg[..., :W]
        pg[..., W + 2] = 0.0
        parts.append(hdr.reshape(ncores, P, 2 * gsz))
        parts.append(pg.reshape(ncores, P, gsz * (W + 3)))
    arrs = np.concatenate(parts, axis=2)
    return np.ascontiguousarray(arrs), L_list, groups, src_cpt


_NC_CACHE = {}


def _get_nc(L_list, groups):
    key = tuple(groups)
    if key not in _NC_CACHE:
        _NC_CACHE[key] = build_nc(L_list, groups)
    return _NC_CACHE[key]


def _unpack_core(yc, groups):
    """[P, 3*TILES] device layout -> [P, tiles, 3] reference layout."""
    ntiles = sum(g[1] for g in groups)
    yt = np.empty((P, ntiles, 3), np.float32)
    for t0, gsz, W in groups:
        pairs = yc[:, 3 * t0 : 3 * t0 + 2 * gsz].reshape(P, gsz, 2)
        yt[:, t0 : t0 + gsz, 0] = pairs[..., 0]  # cpz[bid]
        yt[:, t0 : t0 + gsz, 2] = pairs[..., 1]  # cpz[mp]
        yt[:, t0 : t0 + gsz, 1] = yc[:, 3 * t0 + 2 * gsz : 3 * (t0 + gsz)]
    return yt


def kernel(inputs):
    global LAST_RESULTS
    x = np.ascontiguousarray(np.asarray(inputs), dtype=np.float32)
    assert x.shape == (BTOT, COLS), x.shape

    arrs, L_list, groups, src_cpt = _prepare(x, NCORES, TILES)
    in_maps = [{"inp": np.ascontiguousarray(arrs[c])} for c in range(NCORES)]

    nc = _get_nc(L_list, groups)
    from concourse.bass_utils import run_bass_kernel_spmd

    r = run_bass_kernel_spmd(
        nc, in_maps, core_ids=list(range(NCORES)), trace=TRACE
    )
    LAST_RESULTS = r
    ys = np.stack(
        [
            _unpack_core(np.asarray(r.results[c]["out"]), groups)
            for c in range(NCORES)
        ]
    )
    out = np.empty((NCORES * BPC, 3), dtype=np.float32)
    out[src_cpt.reshape(-1)] = ys.reshape(-1, 3)
    return np.ascontiguousarray(out[:BTOT])


# revision 33
# speedup vs baseline: 2.5046x; 1.1682x over previous
"""Trainium2 Bass kernel for BidPrefix: per-row cumprod + 3-point gather.

Reference semantics (per row b of inputs [B, 302]):
  rates = inputs[b, :300]; bid = int(inputs[b, 300]); mp = int(inputs[b, 301])
  cpz[k] = prod(rates[:k]) (cpz[0] = 1)
  out[b] = [cpz[bid], cpz[mp+1], cpz[mp]]

Strategy: pure data parallel over 8 NeuronCores. Rows are host-sorted by
max(bid, mp) descending and packed 128-per-tile so every tap in tile t
lies below a per-tile bound L[t]. Tiles are grouped into chunks whose
page width W is quantized to 12 fixed levels; per chunk the host packs
two header blocks (bid[gsz], rates[mp][gsz]) and contiguous pages
[mp, 1.0, rates[0:W], 0.0] (N = W+3) in a flat [128, TOT] DRAM layout.

On device, ONE hand-written custom DVE op per W-level (MERGETAP{W}_ANT)
computes BOTH taps in a single pass over each page. Its 6-uop FSM
[seed; boundary; steadyA; steadyB; penultimate; last] runs, per page:
pgidx = -1,0,1,... (so the packed 1.0 gives cp[e] = cpz[e] exactly),
cp = running product, R1 += (pgidx==bid)*cp with bid streamed stride-0
from the header (Src1), and R2 += (pgidx==mp)*cp with mp captured from
the page's first cell into the stage-5 swap flop by the boundary uop.
The page interior is covered by two COUNT-repeat steady uops (the
repeat_cnt field is 8-bit, so W up to 300 is split in half); every
consuming uop writes (write-gated uops hang the engine), so the dst is a
stride-1 junk strip whose page slots N-2 and N-1 receive cpz[bid] (the
penultimate uop selects the R1 delay lane) and cpz[mp] (the last uop
selects ALU_OUT = R2). The trailing 0.0 pad cell guarantees both sums
are complete by slot N-2. Results leave via a strided [P, gsz, 2] DMA of
those slots plus cpz[mp+1] = cpz[mp] * rates[mp], one small Vector
multiply per chunk against the packed rates[mp] block (bit-exact with
the reference's sequential f32 cumprod). bid==0 / mp==0 fall out
naturally (cp[0] = 1). The host does layout only; every multiply happens
on device.
"""

import dataclasses
import sys

if "/opt/trn_rl_repo" not in sys.path:
    sys.path.insert(0, "/opt/trn_rl_repo")

import numpy as np

S = 300
COLS = 302
P = 128
NCORES = 8
TILES = 196
BPC = TILES * P  # 25088 rows per core
BTOT = 200000
BUDGET = 6144  # per-partition f32 slots per chunk
RAMP = [384, 768, 1536, 3072]

# quantized page-width levels (one custom op per level; 5-bit opcode-row
# budget allows 31 - 16 builtins = 15)
LEVELS = [300, 268, 240, 214, 192, 171, 153, 137, 122, 109, 97, 85, 68, 45, 2]

TRACE = False
LAST_RESULTS = None

_MERGETAPS = None


def _get_mergetaps():
    """Register the merged two-tap page ops, one per W level (idempotent)."""
    global _MERGETAPS
    if _MERGETAPS is not None:
        return _MERGETAPS
    import concourse.dve_ops as dve_ops
    from concourse.dve_ops import OPS, DveOp
    from concourse.dve_spec import (
        AluOp, Bin, Scan, Spec, Src0, Src1, Zero, One, eq,
    )
    from concourse.dve_uop import (
        AluInp, DelayInp, DveOpSpec, InpSel, OutPath, OutSel, Trigger,
        UopConfig, UopDpConfig, ENABLE,
    )

    existing = {op.name: op for op in OPS}

    LSRC0, LSRC1, LONE, LA, LB, LC = 0, 1, 2, 3, 4, 5
    D = AluInp.PREV_DELAY_0

    def lane(i):
        return AluInp(int(D) + i)

    def _ref(in0, in1, s0, s1, imm2):
        x = in0.astype(np.float32)
        cpz = np.cumprod(x[..., 1:], axis=-1, dtype=np.float32)
        mp = x[..., 0].astype(np.int64)[..., None]
        bid = np.asarray(in1, np.float32)[..., 0].astype(np.int64)[..., None]
        out = np.zeros(x.shape, np.float32)
        out[..., -2] = np.take_along_axis(cpz, bid, axis=-1)[..., 0]
        out[..., -1] = np.take_along_axis(cpz, mp, axis=-1)[..., 0]
        return out

    def mk_steady():
        u = UopConfig()
        u.enable_input(InpSel.SRC_0, LSRC0 + 1)
        u.enable_input(InpSel.SRC_1, LSRC1 + 1)
        u.enable_input(InpSel.ONE_F32, LONE + 1)
        dp = u.datapath_config
        for st in range(8):
            dp[st].pass_through_delay(LSRC0, LSRC1, LONE, LA, LB, LC)
        # st0 pgidx; st1 cp (capture pgidx->A); st2 eq1 (capture cp->B);
        # st3 v1; st4 R1; st5 eq2 vs swap[mp] (capture R1->C); st6 v2;
        # st7 R2 — R1/R2 are CURR-feedback running sums
        dp[0].enable_alu(AluOp.ADD, AluInp.CURR_ALU_OUT, lane(LONE))
        dp[1].enable_alu(AluOp.MULTIPLY, AluInp.CURR_ALU_OUT, lane(LSRC0))
        dp[1].enable_delay_from_src(DelayInp.PREV_ALU_OUT, LA)
        dp[2].enable_alu(AluOp.IS_EQ, lane(LA), lane(LSRC1))
        dp[2].enable_delay_from_src(DelayInp.PREV_ALU_OUT, LB)
        dp[3].enable_alu(AluOp.MULTIPLY, AluInp.PREV_ALU_OUT, lane(LB))
        dp[4].enable_alu(AluOp.ADD, AluInp.CURR_ALU_OUT, AluInp.PREV_ALU_OUT)
        dp[5].enable_alu(AluOp.IS_EQ, lane(LA), AluInp.CURR_SWAP_OUT)
        dp[5].enable_delay_from_src(DelayInp.PREV_ALU_OUT, LC)
        dp[6].enable_alu(AluOp.MULTIPLY, AluInp.PREV_ALU_OUT, lane(LB))
        dp[7].enable_alu(AluOp.ADD, AluInp.CURR_ALU_OUT, AluInp.PREV_ALU_OUT)
        u.require_inp0 = ENABLE
        u.require_inp1 = ENABLE
        # every consuming uop must write (write-gated uops hang the engine)
        u.enable_output(OutSel.ALU_OUT, OutPath.WR0_LO)
        return u

    def mk_boundary():
        # first element of each page (the [mp] cell): reset the three
        # feedback flops and capture mp into st5's swap flop
        u = mk_steady()
        dp = u.datapath_config
        u.enable_input(InpSel.CONST_0, LC + 1)  # s0 immediate = -1.0
        dp[0].enable_alu(AluOp.BYPASS, lane(LC))
        dp[1].enable_alu(AluOp.BYPASS, lane(LONE))
        dp[4].enable_alu(AluOp.BYPASS, AluInp.PREV_ALU_OUT)
        dp[5] = UopDpConfig()
        dp[5].pass_through_delay(LSRC0, LSRC1, LONE, LA, LB, LC)
        dp[5].enable_alu(AluOp.BYPASS, lane(LSRC0))
        dp[5].swap_enable = ENABLE
        dp[6].enable_alu(AluOp.IS_EQ, lane(LA), AluInp.PREV_ALU_OUT)
        dp[7].enable_alu(AluOp.BYPASS, AluInp.PREV_ALU_OUT)
        u.repeat_count = 1
        u.trigger = (Trigger.SRC_TENSOR_DONE, Trigger.NONE, Trigger.COUNT)
        u.next_uop = (0, 0, 2)
        return u

    def mk_seed():
        # non-consuming entry: reset the feedback flops
        u = UopConfig()
        u.enable_input(InpSel.ONE_F32, LONE + 1)
        u.enable_input(InpSel.CONST_0, LC + 1)
        u.enable_input(InpSel.ZERO, LB + 1)
        dp = u.datapath_config
        for st in range(8):
            dp[st].pass_through_delay(LONE, LB, LC)
        dp[0].enable_alu(AluOp.BYPASS, lane(LC))
        dp[1].enable_alu(AluOp.BYPASS, lane(LONE))
        dp[4].enable_alu(AluOp.BYPASS, lane(LB))
        dp[7].enable_alu(AluOp.BYPASS, lane(LB))
        u.repeat_count = 1
        u.trigger = (Trigger.COUNT, Trigger.NONE, Trigger.NONE)
        u.next_uop = (1, 0, 0)
        return u

    def _uops(W, ver):
        # page N = W+3; boundary 1 elem + steadies W elems + pen 1 + last 1
        ra = (W + 1) // 2
        rb = W - ra
        assert 1 <= ra <= 255 and 1 <= rb <= 255, W
        seed = mk_seed()
        b1 = mk_boundary()
        stA = mk_steady()
        stA.repeat_count = ra
        stA.trigger = (Trigger.SRC_TENSOR_DONE, Trigger.NONE, Trigger.COUNT)
        stA.next_uop = (0, 0, 3)
        stB = mk_steady()
        stB.repeat_count = rb
        stB.trigger = (Trigger.SRC_TENSOR_DONE, Trigger.NONE, Trigger.COUNT)
        stB.next_uop = (0, 0, 4)
        pen = mk_steady()
        pen.repeat_count = 1
        pen.trigger = (Trigger.SRC_TENSOR_DONE, Trigger.NONE, Trigger.COUNT)
        pen.next_uop = (0, 0, 5)
        pen.out[OutPath.WR0_LO] = OutSel(int(OutSel.DELAY_0) + LC)  # R1
        last = mk_steady()
        last.trigger = (
            Trigger.SRC_TENSOR_DONE, Trigger.SUB_DIM_DONE, Trigger.NONE,
        )
        last.next_uop = (0, 1, 0)
        uops = [seed, b1, stA, stB, pen, last]
        for u in uops:
            u.validate(ver)
        return uops

    @dataclasses.dataclass(frozen=True)
    class _RawDveOp(DveOp):
        raw_uops: dict = dataclasses.field(
            default_factory=dict, compare=False, hash=False
        )

        def compile(self, ver):
            sp = DveOpSpec(
                name=self.name,
                opcode=dve_ops.get_dve_sub_opcode(self.name),
                uops=self.raw_uops[ver],
                rd1_en=True,
            )
            sp.validate(ver)
            return sp

    spec_body = eq(
        Scan(AluOp.ADD, One, init=Bin(AluOp.SUBTRACT, Zero, One)), Src1
    ) * Scan(AluOp.MULTIPLY, Src0, init=One)

    taps = {}
    for W in LEVELS:
        name = f"MERGETAP{W}_ANT"
        if name in existing:
            taps[W] = existing[name]
            continue
        spec = Spec(body=spec_body, reference=_ref)
        raw = {ver: _uops(W, ver) for ver in ("v3", "v4")}
        shas = {
            ver: DveOpSpec(name=name, opcode=0, uops=u, rd1_en=True).sha(ver)
            for ver, u in raw.items()
        }
        op = _RawDveOp(name, spec, subdim=True, uops_sha=shas, raw_uops=raw)
        OPS.append(op)
        dve_ops._SUB_OPCODE_FOR_NAME[name] = (
            dve_ops._CUSTOM_DVE_ROW_BASE + len(OPS) - 1
        )
        dve_ops.CUSTOM_DVE_SPECS[name] = spec
        taps[W] = op
    _MERGETAPS = taps
    return taps


def _emit_mergetap(nc, op, out0, in0, in1):
    import concourse.bass_isa as bass_isa
    import concourse.mybir as mybir
    from concourse.dve_ops import get_dve_sub_opcode

    v = nc.vector
    if op.name not in nc.m.ant_custom_dve_ops:
        nc.m.ant_custom_dve_ops = sorted({*nc.m.ant_custom_dve_ops, op.name})
    shape = bass_isa.CustomDveShape.STT
    isa_opcode = nc.isa.Opcode[
        f"NEURON_ISA_TPB_OPCODE_CUSTOM_DVE_ANT_{shape.slot()}"
    ].value
    ins = [
        v.lower_ap(in0, for_isa=True, opt=False),
        v.lower_ap(in1, for_isa=True, opt=False),
        mybir.ImmediateValue(dtype=mybir.dt.float32, value=-1.0),
        mybir.ImmediateValue(dtype=mybir.dt.float32, value=0.0),
    ]
    outs = [v.lower_ap(out0, for_isa=True, opt=False)]
    return v.add_instruction(
        bass_isa.InstCustomDveAnt(
            name=nc.get_next_instruction_name(),
            op_name=op.name,
            rd1_en=True,
            subdim=0x02,
            imm2=0.0,
            shape=shape,
            row=get_dve_sub_opcode(op.name),
            isa_opcode=isa_opcode,
            ins=ins,
            outs=outs,
        )
    )


def _plan_groups(L_list):
    """Chunks of tiles sharing one quantized page width W: walk tiles
    (sorted L desc), W = smallest level >= L[t0]; fill until the budget.
    Small ramp-up budgets let the DVE start early."""
    n = len(L_list)
    groups = []
    t0 = 0
    gi = 0
    while t0 < n:
        budget = RAMP[gi] if gi < len(RAMP) else BUDGET
        W = min(lv for lv in LEVELS if lv >= max(int(L_list[t0]), 1))
        gsz = max(1, budget // (W + 5))
        gsz = min(gsz, n - t0)
        # all tiles in the chunk must fit level W
        for j in range(1, gsz):
            if L_list[t0 + j] > W:
                gsz = j
                break
        rem = n - t0 - gsz
        if 0 < rem < 3:
            gsz = max(1, gsz - (3 - rem))
        groups.append((t0, gsz, W))
        t0 += gsz
        gi += 1
    t0, gsz, W = groups[-1]
    if gsz >= 12:
        groups[-1] = (t0, gsz - 8, W)
        W2 = min(lv for lv in LEVELS if lv >= max(int(L_list[t0 + gsz - 8]), 1))
        W3 = min(lv for lv in LEVELS if lv >= max(int(L_list[t0 + gsz - 2]), 1))
        groups.append((t0 + gsz - 8, 6, W2))
        groups.append((t0 + gsz - 2, 2, W3))
    return groups


def _group_cols(gsz, W):
    """Per-partition f32 slots: 2 header blocks (bid, rates[mp]) +
    contiguous pages [mp, 1.0, rates[0:W], 0.0]."""
    return 2 * gsz + gsz * (W + 3)


def build_nc(L_list, groups=None):
    import concourse.bacc as bacc
    import concourse.mybir as mybir
    from concourse import tile

    f32 = mybir.dt.float32
    A = mybir.AluOpType
    TAPS = _get_mergetaps()

    if groups is None:
        groups = _plan_groups(L_list)
    ntiles = len(L_list)
    offs = [0]
    for _, gsz, W in groups:
        offs.append(offs[-1] + _group_cols(gsz, W))

    nc = bacc.Bacc("TRN2", target_bir_lowering=False, debug=False)
    inp = nc.dram_tensor("inp", [P, offs[-1]], f32, kind="ExternalInput")
    out = nc.dram_tensor("out", [P, ntiles * 3], f32, kind="ExternalOutput")
    vin = inp.ap()
    vout = out.ap()

    with tile.TileContext(nc) as tc:
        with (
            tc.tile_pool(name="raw", bufs=5) as rawp,
            tc.tile_pool(name="junk", bufs=3) as junkp,
            tc.tile_pool(name="res", bufs=4) as resp,
        ):
            prepped = {}

            def prep(gj):
                _, gsz, W = groups[gj]
                g = rawp.tile([P, _group_cols(gsz, W)], f32, tag="raw")
                nc.sync.dma_start(g, vin[:, offs[gj] : offs[gj + 1]])
                prepped[gj] = g

            for gj in range(min(5, len(groups))):
                prep(gj)
            for gi, (t0, gsz, W) in enumerate(groups):
                if gi + 5 < len(groups):
                    prep(gi + 5)
                N = W + 3
                g = prepped.pop(gi)
                pages = g[:, 2 * gsz :].rearrange("p (s w) -> p s w", w=N)
                junk = junkp.tile([P, gsz * N], f32, tag="junk")
                j3 = junk.rearrange("p (s w) -> p s w", w=N)
                _emit_mergetap(
                    nc, TAPS[W],
                    out0=j3,
                    in0=pages,
                    in1=g[:, 0:gsz].unsqueeze(2).broadcast_to([P, gsz, N]),
                )
                # compact the tap pairs into contiguous SBUF (a strided
                # [P, gsz, 2] DMA straight from the junk strip generates
                # 8-byte scatter descriptors at ~6us per transfer), then
                # cpz[mp+1] = cpz[mp] * rates[mp], then ONE contiguous DMA
                res = resp.tile([P, 3 * gsz], f32, tag="res")
                nc.vector.tensor_copy(
                    res[:, 0 : 2 * gsz].rearrange("p (s k) -> p s k", k=2),
                    j3[:, :, N - 2 : N],
                )
                nc.vector.tensor_tensor(
                    res[:, 2 * gsz :], j3[:, :, N - 1], g[:, gsz : 2 * gsz],
                    A.mult,
                )
                nc.scalar.dma_start(vout[:, 3 * t0 : 3 * (t0 + gsz)], res)

    nc.compile()
    return nc


def _prepare(x, ncores, tiles):
    """Sort rows by max(bid, mp) desc, pack into per-core flat page layout.

    Returns (arrs [ncores, P, TOT], L_list, groups, src_cpt)."""
    bpc = tiles * P
    npad = bpc * ncores - x.shape[0]
    assert npad >= 0
    if npad:
        padrows = np.zeros((npad, COLS), dtype=np.float32)
        padrows[:, :S] = 1.0
        xp = np.concatenate([x, padrows], axis=0)
    else:
        xp = x

    key = np.maximum(xp[:, S], xp[:, S + 1]).astype(np.int64)
    order = np.argsort(-key, kind="stable")
    nblocks = ncores * tiles
    src = order.reshape(nblocks, P).reshape(tiles, ncores, P)
    src_cpt = np.ascontiguousarray(src.transpose(1, 2, 0))  # [core, p, t]

    block_max = key[order].reshape(nblocks, P)[:, 0]
    L_list = np.maximum(block_max.reshape(tiles, ncores).max(axis=1), 1)
    L_list = [int(v) for v in L_list]
    groups = _plan_groups(L_list)

    rows = xp[src_cpt]  # [ncores, P, tiles, COLS]
    parts = []
    for t0, gsz, W in groups:
        rg = rows[:, :, t0 : t0 + gsz, :]
        hdr = np.empty((ncores, P, 2, gsz), dtype=np.float32)
        hdr[:, :, 0] = rg[..., S]  # bid
        mp_i = rg[..., S + 1].astype(np.int64)[..., None]
        hdr[:, :, 1] = np.take_along_axis(rg[..., :S], mp_i, axis=-1)[..., 0]
        pg = np.empty((ncores, P, gsz, W + 3), dtype=np.float32)
        pg[..., 0] = rg[..., S + 1]  # mp
        pg[..., 1] = 1.0
        pg[..., 2 : 2 + W] = rg[..., :W]
        pg[..., W + 2] = 0.0
        parts.append(hdr.reshape(ncores, P, 2 * gsz))
        parts.append(pg.reshape(ncores, P, gsz * (W + 3)))
    arrs = np.concatenate(parts, axis=2)
    return np.ascontiguousarray(arrs), L_list, groups, src_cpt


_NC_CACHE = {}


def _get_nc(L_list, groups):
    key = tuple(groups)
    if key not in _NC_CACHE:
        _NC_CACHE[key] = build_nc(L_list, groups)
    return _NC_CACHE[key]


def _unpack_core(yc, groups):
    """[P, 3*TILES] device layout -> [P, tiles, 3] reference layout."""
    ntiles = sum(g[1] for g in groups)
    yt = np.empty((P, ntiles, 3), np.float32)
    for t0, gsz, W in groups:
        pairs = yc[:, 3 * t0 : 3 * t0 + 2 * gsz].reshape(P, gsz, 2)
        yt[:, t0 : t0 + gsz, 0] = pairs[..., 0]  # cpz[bid]
        yt[:, t0 : t0 + gsz, 2] = pairs[..., 1]  # cpz[mp]
        yt[:, t0 : t0 + gsz, 1] = yc[:, 3 * t0 + 2 * gsz : 3 * (t0 + gsz)]
    return yt


def kernel(inputs):
    global LAST_RESULTS
    x = np.ascontiguousarray(np.asarray(inputs), dtype=np.float32)
    assert x.shape == (BTOT, COLS), x.shape

    arrs, L_list, groups, src_cpt = _prepare(x, NCORES, TILES)
    in_maps = [{"inp": np.ascontiguousarray(arrs[c])} for c in range(NCORES)]

    nc = _get_nc(L_list, groups)
    from concourse.bass_utils import run_bass_kernel_spmd

    r = run_bass_kernel_spmd(
        nc, in_maps, core_ids=list(range(NCORES)), trace=TRACE
    )
    LAST_RESULTS = r
    ys = np.stack(
        [
            _unpack_core(np.asarray(r.results[c]["out"]), groups)
            for c in range(NCORES)
        ]
    )
    out = np.empty((NCORES * BPC, 3), dtype=np.float32)
    out[src_cpt.reshape(-1)] = ys.reshape(-1, 3)
    return np.ascontiguousarray(out[:BTOT])
